# revision 15
# baseline (speedup 1.0000x reference)
"""
CIN (Compressed Interaction Network) kernel for Trainium2, 8 NeuronCores.

Problem (hardcoded shapes):
  x: [4096, 32, 64] fp32
  w0: [128, 1024] fp32, b0: [128]   (layer 0: z0 = outer(x, x) per (b, d))
  w1: [128, 2048] fp32, b1: [128]   (layer 1: z1 = outer(hidden, x))
  out: [4096, 192] fp32 = concat(relu(y0)[64:], relu(y1)).sum(dim=d)

Sharding: pure data parallel over batch, 512 samples per core.

Per-core layout ("tokens" t = (b, d), d-innermost, T = 512*64 = 32768):
  - XrepFull [128, T] bf16 in SBUF: x rows tiled 4x along partitions
    (partition p holds x[f = p % 32, :]).  Rows 0:32 are x itself.
  - Z c-tiles are [128, Tc]: channel c = 128 g + p maps to
    (h = 4 g + p // 32, f = p % 32); both layers use identity channel order,
    so lhsT is just w.T sliced per 128 rows.
  - Hexp (the h-side broadcast) is a single DMA per (chunk, g): source AP
    [[1, 4], [0, 32], [1, Tc]] reads 4 source rows, 32 repeats each ->
    128 dest partitions.
  - Z = XrepFull * Hexp on VectorE (bf16 tensor_tensor, 2x mode).
  - Matmul accumulates over g in PSUM; ScalarE applies bias+ReLU.
  - d-sums via ScalarE activation accum_out per 64-token group.
"""

import os
import sys

import numpy as np
import ml_dtypes

sys.path.insert(0, "/opt/trn_rl_repo")

B_FULL = 4096
N_CORES = 8
BS = B_FULL // N_CORES  # 512 samples per core
F = 32
D = 64
T = BS * D  # 32768 tokens per core
CHUNK = 1024  # tokens per chunk (16 samples)
NCHUNK = T // CHUNK
SPC = CHUNK // D  # samples per chunk = 16
O = 128  # conv output channels per layer
H1 = 64  # hidden rows fed to layer 1
G0 = 8   # layer-0 c-tiles (1024 channels)
G1 = 16  # layer-1 c-tiles (2048 channels)

_CACHE = {}


def _build_nc(BS=BS, CHUNK=CHUNK):
    import concourse.bass as bass
    import concourse.tile as tile
    from concourse import bacc, mybir

    T = BS * D
    NCHUNK = T // CHUNK
    SPC = CHUNK // D

    bf16 = mybir.dt.bfloat16
    f32 = mybir.dt.float32
    Relu = mybir.ActivationFunctionType.Relu

    nc = bacc.Bacc(None, target_bir_lowering=False)

    # x pre-transposed, pre-cast, pre-replicated 4x on host: [128, T] bf16
    xt = nc.dram_tensor("xt", [128, T], bf16, kind="ExternalInput")
    w0t = nc.dram_tensor("w0t", [G0 * 128, O], bf16, kind="ExternalInput")
    w1t = nc.dram_tensor("w1t", [G1 * 128, O], bf16, kind="ExternalInput")
    b0 = nc.dram_tensor("b0", [O, 1], f32, kind="ExternalInput")
    b1 = nc.dram_tensor("b1", [O, 1], f32, kind="ExternalInput")
    out0 = nc.dram_tensor("out0", [O - H1, BS], f32, kind="ExternalOutput")
    out1 = nc.dram_tensor("out1", [O, BS], f32, kind="ExternalOutput")

    def bcast_src(ap4):
        # [4, Tc] slice -> [[1,4],[0,32],[1,Tc]]: row j repeated to 32
        # consecutive dest partitions.
        assert len(ap4.ap) == 2
        return bass.AP(
            tensor=ap4.tensor,
            offset=ap4.offset,
            ap=[list(ap4.ap[0]), [0, 32], list(ap4.ap[1])],
        )

    with tile.TileContext(nc) as tc:
        with (
            tc.tile_pool(name="singles", bufs=1) as singles,
            tc.tile_pool(name="hx", bufs=6) as hxp,
            tc.tile_pool(name="z", bufs=6) as zp,
            tc.tile_pool(name="ysb", bufs=3) as ysbp,
            tc.tile_pool(name="psum0", bufs=2, space="PSUM") as pp0,
            tc.tile_pool(name="psum1", bufs=2, space="PSUM") as pp1,
        ):
            xr = singles.tile([128, T], bf16)
            w0s = singles.tile([128, G0, O], bf16)
            w1s = singles.tile([128, G1, O], bf16)
            b0s = singles.tile([O, 1], f32)
            b1s = singles.tile([O, 1], f32)
            oacc0 = singles.tile([O, BS], f32)
            oacc1 = singles.tile([O, BS], f32)
            dum0 = singles.tile([64, D], bf16)
            dum1 = singles.tile([O, D], bf16)

            nc.sync.dma_start(out=xr[:], in_=xt[:])

            nc.sync.dma_start(out=w0s[:], in_=w0t.rearrange("(g k) m -> k g m", k=128))
            nc.sync.dma_start(out=w1s[:], in_=w1t.rearrange("(g k) m -> k g m", k=128))
            nc.sync.dma_start(out=b0s[:], in_=b0[:])
            nc.sync.dma_start(out=b1s[:], in_=b1[:])

            for k in range(NCHUNK):
                sl = slice(k * CHUNK, (k + 1) * CHUNK)
                xsl = xr[:, sl]

                # ---- layer 0 ----
                y0p = pp0.tile([O, CHUNK], f32)
                for g in range(G0):
                    hx = hxp.tile([128, CHUNK], bf16)
                    nc.sync.dma_start(out=hx[:], in_=bcast_src(xr[4 * g : 4 * g + 4, sl]))
                    z = zp.tile([128, CHUNK], bf16)
                    nc.vector.tensor_mul(z[:], xsl, hx[:])
                    for s in range(CHUNK // 512):
                        cs = slice(s * 512, (s + 1) * 512)
                        nc.tensor.matmul(
                            y0p[:, cs], w0s[:, g, :], z[:, cs],
                            start=(g == 0), stop=(g == G0 - 1),
                        )

                # hidden rows -> SBUF bf16 (bias + relu)
                y0sb = ysbp.tile([H1, CHUNK], bf16)
                nc.scalar.activation(y0sb[:], y0p[0:H1, :], Relu, bias=b0s[0:H1])
                # direct rows: bias + relu + per-sample d-sum
                for i in range(SPC):
                    ds = slice(i * D, (i + 1) * D)
                    col = k * SPC + i
                    nc.scalar.activation(
                        dum0[:], y0p[H1:O, ds], Relu, bias=b0s[H1:O],
                        accum_out=oacc0[H1:O, col : col + 1],
                    )

                # ---- layer 1 ----
                y1p = pp1.tile([O, CHUNK], f32)
                for g in range(G1):
                    hx = hxp.tile([128, CHUNK], bf16)
                    nc.sync.dma_start(out=hx[:], in_=bcast_src(y0sb[4 * g : 4 * g + 4, :]))
                    z = zp.tile([128, CHUNK], bf16)
                    nc.vector.tensor_mul(z[:], xsl, hx[:])
                    for s in range(CHUNK // 512):
                        cs = slice(s * 512, (s + 1) * 512)
                        nc.tensor.matmul(
                            y1p[:, cs], w1s[:, g, :], z[:, cs],
                            start=(g == 0), stop=(g == G1 - 1),
                        )

                for i in range(SPC):
                    ds = slice(i * D, (i + 1) * D)
                    col = k * SPC + i
                    nc.scalar.activation(
                        dum1[:], y1p[:, ds], Relu, bias=b1s[:],
                        accum_out=oacc1[:, col : col + 1],
                    )

            nc.sync.dma_start(out=out0[:], in_=oacc0[H1:O, :])
            nc.sync.dma_start(out=out1[:], in_=oacc1[:])

    nc.finalize()
    return nc


def _get_nc():
    if "nc" not in _CACHE:
        _CACHE["nc"] = _build_nc()
    return _CACHE["nc"]


def kernel(cin_inputs, w0, b0, w1, b1, _trace=False):
    from concourse.bass_utils import run_bass_kernel_spmd

    x = np.asarray(cin_inputs, dtype=np.float32)
    assert x.shape == (B_FULL, F, D)
    bf = ml_dtypes.bfloat16
    # [B, F, D] -> per-core [F, BS*D] bf16, tiled 4x along partitions
    xt_all = np.ascontiguousarray(
        x.reshape(N_CORES, BS, F, D).transpose(0, 2, 1, 3)
    ).astype(bf).reshape(N_CORES, F, BS * D)
    xt_all = np.ascontiguousarray(np.tile(xt_all, (1, 4, 1)))
    w0t = np.ascontiguousarray(np.asarray(w0, dtype=np.float32).T).astype(bf)
    w1t = np.ascontiguousarray(np.asarray(w1, dtype=np.float32).T).astype(bf)
    b0c = np.asarray(b0, dtype=np.float32).reshape(O, 1).copy()
    b1c = np.asarray(b1, dtype=np.float32).reshape(O, 1).copy()

    nc = _get_nc()
    in_maps = []
    for i in range(N_CORES):
        in_maps.append(
            {
                "xt": xt_all[i],
                "w0t": w0t,
                "w1t": w1t,
                "b0": b0c,
                "b1": b1c,
            }
        )
    res = run_bass_kernel_spmd(
        nc, in_maps, core_ids=list(range(N_CORES)), trace=_trace
    )
    outs = []
    for r in res.results:
        # [192, 512] -> [512, 192]
        o = np.concatenate([r["out0"], r["out1"]], axis=0).T
        outs.append(o)
    full = np.concatenate(outs, axis=0).astype(np.float32)
    if _trace:
        return full, res
    return full


if __name__ == "__main__":
    rng = np.random.default_rng(0)
    x = rng.standard_normal((B_FULL, F, D), dtype=np.float32)
    w0 = (rng.random((128, 1024), dtype=np.float32) - 0.5) * 0.0625
    b0 = (rng.random(128, dtype=np.float32) - 0.5) * 0.0625
    w1 = (rng.random((128, 2048), dtype=np.float32) - 0.5) * 0.0442
    b1 = (rng.random(128, dtype=np.float32) - 0.5) * 0.0442
    out = kernel(x, w0, b0, w1, b1)
    print("kernel out", out.shape, out.dtype, float(np.abs(out).mean()))


# revision 17
# speedup vs baseline: 2.8238x; 2.8238x over previous
"""
CIN (Compressed Interaction Network) kernel for Trainium2, 8 NeuronCores.

Problem (hardcoded):
  x: [4096, 32, 64] fp32; w0: [128, 1024]; b0: [128]; w1: [128, 2048]; b1: [128]
  out: [4096, 192] = concat(relu(y0)[:, 64:], relu(y1)).sum(d)
  y0 = w0 @ vec(x (x) x) per (b, d) token; y1 = w1 @ vec(hidden (x) x).

Sharding: data parallel over batch, 512 samples/core, tokens t=(b,d), T=32768.

Per-core pipeline (pair = 2048 tokens):
  - xr [128, 2048] bf16: x rows tiled 4x across partitions (from host input xt).
  - Broadcast tiles Hexp_g (channel c = 128g + p -> (h = 4g + p//32, f = p%32))
    are built ON THE PE as row-tiled one-hot matmuls: two concurrent 64x128
    tiles (tile_position rows 0 / 64) write separate PSUM banks; ScalarE
    evacuates pairs to SBUF bf16.
  - Z_g = xr * Hexp_g on VectorE (bf16 tensor_tensor, 2x mode).
  - W contraction accumulates over g in PSUM (start/stop flags).
  - ScalarE: bias+ReLU evac; VectorE: per-sample d-sum reduce.
"""

import sys

import numpy as np
import ml_dtypes

sys.path.insert(0, "/opt/trn_rl_repo")

B_FULL = 4096
N_CORES = 8
BS = B_FULL // N_CORES  # 512
F = 32
D = 64
T = BS * D  # 32768
PAIR = 2048  # tokens per pair (32 samples)
HALF = 1024
O = 128
H1 = 64
G0 = 8
G1 = 16

_CACHE = {}


def _build_nc(BS=BS, PAIR=PAIR):
    import concourse.bass as bass
    import concourse.tile as tile
    from concourse import bacc, mybir

    T = BS * D
    NPAIR = T // PAIR
    HALF = PAIR // 2
    SPP = PAIR // D  # samples per pair

    bf16 = mybir.dt.bfloat16
    f32 = mybir.dt.float32
    Relu = mybir.ActivationFunctionType.Relu
    X = mybir.AxisListType.X
    ADD = mybir.AluOpType.add

    nc = bacc.Bacc(None, target_bir_lowering=False)

    xt = nc.dram_tensor("xt", [128, T], bf16, kind="ExternalInput")
    w0t = nc.dram_tensor("w0t", [G0 * 128, O], bf16, kind="ExternalInput")
    w1t = nc.dram_tensor("w1t", [G1 * 128, O], bf16, kind="ExternalInput")
    sel0 = nc.dram_tensor("sel0", [128, G0, 128], bf16, kind="ExternalInput")
    sel1 = nc.dram_tensor("sel1", [128, G1, 128], bf16, kind="ExternalInput")
    b0 = nc.dram_tensor("b0", [O, 1], f32, kind="ExternalInput")
    b1 = nc.dram_tensor("b1", [O, 1], f32, kind="ExternalInput")
    out0 = nc.dram_tensor("out0", [O - H1, BS], f32, kind="ExternalOutput")
    out1 = nc.dram_tensor("out1", [O, BS], f32, kind="ExternalOutput")

    with tile.TileContext(nc) as tc:
        with (
            tc.tile_pool(name="singles", bufs=1) as singles,
            tc.tile_pool(name="xrp", bufs=3) as xrp,
            tc.tile_pool(name="hx", bufs=4) as hxp,
            tc.tile_pool(name="z", bufs=26) as zp,
            tc.tile_pool(name="ysb", bufs=3) as ysbp,
            tc.tile_pool(name="hdup", bufs=3) as hdupp,
            tc.tile_pool(name="hp", bufs=2, space="PSUM") as hpp,
            tc.tile_pool(name="py0", bufs=1, space="PSUM") as py0p,
            tc.tile_pool(name="py1", bufs=1, space="PSUM") as py1p,
        ):
            w0s = singles.tile([128, G0, O], bf16)
            w1s = singles.tile([128, G1, O], bf16)
            s0s = singles.tile([128, G0, 128], bf16)
            s1s = singles.tile([128, G1, 128], bf16)
            b0s = singles.tile([O, 1], f32)
            b1s = singles.tile([O, 1], f32)
            oacc0 = singles.tile([O, BS], f32)
            oacc1 = singles.tile([O, BS], f32)

            nc.gpsimd.dma_start(out=w0s[:], in_=w0t.rearrange("(g k) m -> k g m", k=128))
            nc.gpsimd.dma_start(out=w1s[:], in_=w1t.rearrange("(g k) m -> k g m", k=128))
            nc.gpsimd.dma_start(out=s0s[:], in_=sel0[:])
            nc.gpsimd.dma_start(out=s1s[:], in_=sel1[:])
            nc.gpsimd.dma_start(out=b0s[:], in_=b0[:])
            nc.gpsimd.dma_start(out=b1s[:], in_=b1[:])

            for P in range(NPAIR):
                sl = slice(P * PAIR, (P + 1) * PAIR)
                xr = xrp.tile([128, PAIR], bf16)
                nc.gpsimd.dma_start(out=xr[:], in_=xt[:, sl])

                def bc_layer(sel_sb, src0, src64, G):
                    """Row-tiled one-hot matmuls -> hx tiles [128, 2, PAIR].

                    g even runs on PE rows 0-63 reading src0, g odd on rows
                    64-127 reading src64. Returns list of (g -> AP of Hexp_g).
                    """
                    hxs = []
                    for gp in range(G // 2):
                        hx2 = hxp.tile([128, 2, PAIR], bf16)
                        for s in range(PAIR // 512):
                            cs = slice(s * 512, (s + 1) * 512)
                            hp = hpp.tile([128, HALF], f32)
                            ga, gb = 2 * gp, 2 * gp + 1
                            nc.tensor.matmul(
                                hp[:, 0:512], sel_sb[0:64, ga, :], src0[:, cs],
                                start=True, stop=True, tile_position=(0, 0),
                            )
                            nc.tensor.matmul(
                                hp[:, 512:1024], sel_sb[64:128, gb, :], src64[:, cs],
                                start=True, stop=True, tile_position=(64, 0),
                            )
                            nc.scalar.activation(
                                hx2[:, :, cs],
                                hp[:].rearrange("p (j c) -> p j c", j=2),
                                mybir.ActivationFunctionType.Copy,
                            )
                        hxs.append(hx2)
                    return hxs

                # ---- layer 0 ----
                hx0 = bc_layer(s0s, xr[0:64, :], xr[64:128, :], G0)
                z0 = []
                for g in range(G0):
                    z = zp.tile([128, PAIR], bf16)
                    nc.vector.tensor_mul(z[:], xr[:], hx0[g // 2][:, g % 2, :])
                    z0.append(z)
                y0sb = ysbp.tile([128, PAIR], bf16)
                for h in range(2):
                    y0p = py0p.tile([O, HALF], f32)
                    for g in range(G0):
                        for s in range(2):
                            cs = slice(h * HALF + s * 512, h * HALF + (s + 1) * 512)
                            ps = slice(s * 512, (s + 1) * 512)
                            nc.tensor.matmul(
                                y0p[:, ps], w0s[:, g, :], z0[g][:, cs],
                                start=(g == 0), stop=(g == G0 - 1),
                            )
                    nc.scalar.activation(
                        y0sb[:, h * HALF : (h + 1) * HALF], y0p[:], Relu, bias=b0s[:]
                    )
                nc.vector.tensor_reduce(
                    oacc0[H1:O, P * SPP : (P + 1) * SPP],
                    y0sb[H1:O, :].rearrange("p (b d) -> p b d", d=D),
                    axis=X, op=ADD,
                )

                # duplicate hidden rows into partitions 64:128 for T8 reads
                hdup = hdupp.tile([128, PAIR], bf16)
                nc.gpsimd.dma_start(out=hdup[64:128, :], in_=y0sb[0:64, :])

                # ---- layer 1 ----
                hx1 = bc_layer(s1s, y0sb[0:64, :], hdup[64:128, :], G1)
                z1 = []
                for g in range(G1):
                    z = zp.tile([128, PAIR], bf16)
                    nc.vector.tensor_mul(z[:], xr[:], hx1[g // 2][:, g % 2, :])
                    z1.append(z)
                y1sb = ysbp.tile([128, PAIR], bf16)
                for h in range(2):
                    y1p = py1p.tile([O, HALF], f32)
                    for g in range(G1):
                        for s in range(2):
                            cs = slice(h * HALF + s * 512, h * HALF + (s + 1) * 512)
                            ps = slice(s * 512, (s + 1) * 512)
                            nc.tensor.matmul(
                                y1p[:, ps], w1s[:, g, :], z1[g][:, cs],
                                start=(g == 0), stop=(g == G1 - 1),
                            )
                    nc.scalar.activation(
                        y1sb[:, h * HALF : (h + 1) * HALF], y1p[:], Relu, bias=b1s[:]
                    )
                nc.vector.tensor_reduce(
                    oacc1[:, P * SPP : (P + 1) * SPP],
                    y1sb[:].rearrange("p (b d) -> p b d", d=D),
                    axis=X, op=ADD,
                )

            nc.gpsimd.dma_start(out=out0[:], in_=oacc0[H1:O, :])
            nc.gpsimd.dma_start(out=out1[:], in_=oacc1[:])

    nc.finalize()
    return nc


def _get_nc():
    if "nc" not in _CACHE:
        _CACHE["nc"] = _build_nc()
    return _CACHE["nc"]


def make_sels():
    sel0 = np.zeros((128, G0, 128), np.float32)
    for g in range(G0):
        base = 64 * (g % 2)
        for p in range(128):
            sel0[base + 4 * g + p // 32, g, p] = 1.0
    sel1 = np.zeros((128, G1, 128), np.float32)
    for g in range(G1):
        base = 64 * (g % 2)
        for p in range(128):
            sel1[base + 4 * g + p // 32, g, p] = 1.0
    bf = ml_dtypes.bfloat16
    return sel0.astype(bf), sel1.astype(bf)


def kernel(cin_inputs, w0, b0, w1, b1, _trace=False):
    from concourse.bass_utils import run_bass_kernel_spmd

    x = np.asarray(cin_inputs, dtype=np.float32)
    assert x.shape == (B_FULL, F, D)
    bf = ml_dtypes.bfloat16
    # [B, F, D] -> per-core [F, BS*D] bf16, tiled 4x along partitions
    xt_all = np.ascontiguousarray(
        x.reshape(N_CORES, BS, F, D).transpose(0, 2, 1, 3)
    ).astype(bf).reshape(N_CORES, F, BS * D)
    xt_all = np.ascontiguousarray(np.tile(xt_all, (1, 4, 1)))
    w0t = np.ascontiguousarray(np.asarray(w0, dtype=np.float32).T).astype(bf)
    w1t = np.ascontiguousarray(np.asarray(w1, dtype=np.float32).T).astype(bf)
    b0c = np.asarray(b0, dtype=np.float32).reshape(O, 1).copy()
    b1c = np.asarray(b1, dtype=np.float32).reshape(O, 1).copy()
    s0, s1 = make_sels()

    nc = _get_nc()
    in_maps = []
    for i in range(N_CORES):
        in_maps.append(
            {
                "xt": xt_all[i], "w0t": w0t, "w1t": w1t,
                "sel0": s0, "sel1": s1, "b0": b0c, "b1": b1c,
            }
        )
    res = run_bass_kernel_spmd(nc, in_maps, core_ids=list(range(N_CORES)), trace=_trace)
    outs = []
    for r in res.results:
        o = np.concatenate([r["out0"], r["out1"]], axis=0).T
        outs.append(o)
    full = np.concatenate(outs, axis=0).astype(np.float32)
    if _trace:
        return full, res
    return full


# revision 26
# speedup vs baseline: 3.3423x; 1.1836x over previous
"""
CIN (Compressed Interaction Network) kernel for Trainium2, 8 NeuronCores.

Problem (hardcoded):
  x: [4096, 32, 64] fp32; w0: [128, 1024]; b0: [128]; w1: [128, 2048]; b1: [128]
  out: [4096, 192] = concat(relu(y0)[:, 64:], relu(y1)).sum(d)
  y0 = w0 @ vec(x (x) x) per (b, d) token; y1 = w1 @ vec(hidden (x) x).

Sharding: data parallel over batch, 512 samples/core, tokens t=(b,d), T=32768.

Per-core pipeline (pair = 2048 tokens):
  - xr [128, 2048] bf16: x rows tiled 4x across partitions (from host input xt).
  - Broadcast tiles Hexp_g (channel c = 128g + p -> (h = 4g + p//32, f = p%32))
    are built ON THE PE as row-tiled one-hot matmuls: two concurrent 64x128
    tiles (tile_position rows 0 / 64) write separate PSUM banks; ScalarE
    evacuates pairs to SBUF bf16.
  - Z_g = xr * Hexp_g on VectorE (bf16 tensor_tensor, 2x mode).
  - W contraction accumulates over g in PSUM (start/stop flags).
  - ScalarE: bias+ReLU evac; VectorE: per-sample d-sum reduce.
"""

import sys

import numpy as np
import ml_dtypes

sys.path.insert(0, "/opt/trn_rl_repo")

B_FULL = 4096
N_CORES = 8
BS = B_FULL // N_CORES  # 512
F = 32
D = 64
T = BS * D  # 32768
PAIR = 2048  # tokens per pair (32 samples)
HALF = 1024
O = 128
H1 = 64
G0 = 8
G1 = 16

_CACHE = {}


def _build_nc(BS=BS, PAIR=PAIR):
    import concourse.bass as bass
    import concourse.tile as tile
    from concourse import bacc, mybir

    T = BS * D
    NPAIR = T // PAIR
    HALF = PAIR // 2
    SPP = PAIR // D  # samples per pair

    bf16 = mybir.dt.bfloat16
    f32 = mybir.dt.float32
    Relu = mybir.ActivationFunctionType.Relu
    X = mybir.AxisListType.X
    ADD = mybir.AluOpType.add

    nc = bacc.Bacc(None, target_bir_lowering=False)

    xt = nc.dram_tensor("xt", [128, T], bf16, kind="ExternalInput")
    # host-expanded layer-0 H side: row 128g+p = x[4g + p//32]
    xe0 = nc.dram_tensor("xe0", [G0 * 128, T], bf16, kind="ExternalInput")
    w0t = nc.dram_tensor("w0t", [G0 * 128, O], bf16, kind="ExternalInput")
    w1t = nc.dram_tensor("w1t", [G1 * 128, O], bf16, kind="ExternalInput")
    sel1 = nc.dram_tensor("sel1", [128, G1, 128], bf16, kind="ExternalInput")
    b0 = nc.dram_tensor("b0", [O, 1], f32, kind="ExternalInput")
    b1 = nc.dram_tensor("b1", [O, 1], f32, kind="ExternalInput")
    out0 = nc.dram_tensor("out0", [O - H1, BS], f32, kind="ExternalOutput")
    out1 = nc.dram_tensor("out1", [O, BS], f32, kind="ExternalOutput")

    with tile.TileContext(nc) as tc:
        with (
            tc.tile_pool(name="singles", bufs=1) as singles,
            tc.tile_pool(name="xrp", bufs=3) as xrp,
            tc.tile_pool(name="hx", bufs=3) as hxp,
            tc.tile_pool(name="hx0", bufs=3) as hx0p,
            tc.tile_pool(name="z", bufs=24) as zp,
            tc.tile_pool(name="ysb", bufs=3) as ysbp,
            tc.tile_pool(name="hdup", bufs=2) as hdupp,
            tc.tile_pool(name="hp", bufs=2, space="PSUM") as hpp,
            tc.tile_pool(name="py0", bufs=1, space="PSUM") as py0p,
            tc.tile_pool(name="py1", bufs=1, space="PSUM") as py1p,
        ):
            w0s = singles.tile([128, G0, O], bf16)
            w1s = singles.tile([128, G1, O], bf16)
            s1s = singles.tile([128, G1, 128], bf16)
            b0s = singles.tile([O, 1], f32)
            b1s = singles.tile([O, 1], f32)
            oacc0 = singles.tile([O, BS], f32)
            oacc1 = singles.tile([O, BS], f32)

            nc.gpsimd.dma_start(out=w0s[:], in_=w0t.rearrange("(g k) m -> k g m", k=128))
            nc.gpsimd.dma_start(out=w1s[:], in_=w1t.rearrange("(g k) m -> k g m", k=128))
            nc.gpsimd.dma_start(out=s1s[:], in_=sel1[:])
            nc.gpsimd.dma_start(out=b0s[:], in_=b0[:])
            nc.gpsimd.dma_start(out=b1s[:], in_=b1[:])

            for P in range(NPAIR):
                sl = slice(P * PAIR, (P + 1) * PAIR)
                xr = xrp.tile([128, PAIR], bf16)
                nc.gpsimd.dma_start(out=xr[:], in_=xt[:, sl])

                def bc_layer(sel_sb, src0, src64, G, evac_dve):
                    """Row-tiled one-hot matmuls -> hx tiles [128, 2, PAIR].

                    g even runs on PE rows 0-63 reading src0, g odd on rows
                    64-127 reading src64. Every evac_dve-th evacuation goes to
                    VectorE instead of ScalarE to balance engine load.
                    """
                    hxs = []
                    k = 0
                    for gp in range(G // 2):
                        hx2 = hxp.tile([128, 2, PAIR], bf16)
                        for s in range(PAIR // 512):
                            cs = slice(s * 512, (s + 1) * 512)
                            hp = hpp.tile([128, HALF], f32)
                            ga, gb = 2 * gp, 2 * gp + 1
                            nc.tensor.matmul(
                                hp[:, 0:512], sel_sb[0:64, ga, :], src0[:, cs],
                                start=True, stop=True, tile_position=(0, 0),
                            )
                            nc.tensor.matmul(
                                hp[:, 512:1024], sel_sb[64:128, gb, :], src64[:, cs],
                                start=True, stop=True, tile_position=(64, 0),
                            )
                            eng = nc.vector if (k % evac_dve == evac_dve - 1) else nc.scalar
                            k += 1
                            if eng is nc.vector:
                                nc.vector.tensor_copy(
                                    hx2[:, :, cs],
                                    hp[:].rearrange("p (j c) -> p j c", j=2),
                                )
                            else:
                                nc.scalar.activation(
                                    hx2[:, :, cs],
                                    hp[:].rearrange("p (j c) -> p j c", j=2),
                                    mybir.ActivationFunctionType.Copy,
                                )
                        hxs.append(hx2)
                    return hxs

                # ---- layer 0: H side comes pre-expanded from HBM ----
                z0 = []
                for g in range(G0):
                    hx = hx0p.tile([128, PAIR], bf16)
                    nc.gpsimd.dma_start(out=hx[:], in_=xe0[128 * g : 128 * (g + 1), sl])
                    z = zp.tile([128, PAIR], bf16)
                    nc.vector.tensor_mul(z[:], xr[:], hx[:])
                    z0.append(z)
                y0sb = ysbp.tile([128, PAIR], bf16)
                for h in range(2):
                    y0p = py0p.tile([O, HALF], f32)
                    for g in range(G0):
                        for s in range(2):
                            cs = slice(h * HALF + s * 512, h * HALF + (s + 1) * 512)
                            ps = slice(s * 512, (s + 1) * 512)
                            nc.tensor.matmul(
                                y0p[:, ps], w0s[:, g, :], z0[g][:, cs],
                                start=(g == 0), stop=(g == G0 - 1),
                            )
                    nc.scalar.activation(
                        y0sb[:, h * HALF : (h + 1) * HALF], y0p[:], Relu, bias=b0s[:]
                    )
                nc.vector.tensor_reduce(
                    oacc0[H1:O, P * SPP : (P + 1) * SPP],
                    y0sb[H1:O, :].rearrange("p (b d) -> p b d", d=D),
                    axis=X, op=ADD,
                )

                # duplicate hidden rows into partitions 64:128 for T8 reads
                hdup = hdupp.tile([128, PAIR], bf16)
                nc.gpsimd.dma_start(out=hdup[64:128, :], in_=y0sb[0:64, :])

                # ---- layer 1 ----
                hx1 = bc_layer(s1s, y0sb[0:64, :], hdup[64:128, :], G1, evac_dve=6)
                z1 = []
                for g in range(G1):
                    z = zp.tile([128, PAIR], bf16)
                    nc.vector.tensor_mul(z[:], xr[:], hx1[g // 2][:, g % 2, :])
                    z1.append(z)
                y1sb = ysbp.tile([128, PAIR], bf16)
                for h in range(2):
                    y1p = py1p.tile([O, HALF], f32)
                    for g in range(G1):
                        for s in range(2):
                            cs = slice(h * HALF + s * 512, h * HALF + (s + 1) * 512)
                            ps = slice(s * 512, (s + 1) * 512)
                            nc.tensor.matmul(
                                y1p[:, ps], w1s[:, g, :], z1[g][:, cs],
                                start=(g == 0), stop=(g == G1 - 1),
                            )
                    nc.scalar.activation(
                        y1sb[:, h * HALF : (h + 1) * HALF], y1p[:], Relu, bias=b1s[:]
                    )
                nc.vector.tensor_reduce(
                    oacc1[:, P * SPP : (P + 1) * SPP],
                    y1sb[:].rearrange("p (b d) -> p b d", d=D),
                    axis=X, op=ADD,
                )

            nc.gpsimd.dma_start(out=out0[:], in_=oacc0[H1:O, :])
            nc.gpsimd.dma_start(out=out1[:], in_=oacc1[:])

    nc.finalize()
    return nc


def _get_nc():
    if "nc" not in _CACHE:
        _CACHE["nc"] = _build_nc()
    return _CACHE["nc"]


def make_sels():
    sel1 = np.zeros((128, G1, 128), np.float32)
    for g in range(G1):
        base = 64 * (g % 2)
        for p in range(128):
            sel1[base + 4 * g + p // 32, g, p] = 1.0
    bf = ml_dtypes.bfloat16
    return sel1.astype(bf)


def kernel(cin_inputs, w0, b0, w1, b1, _trace=False):
    from concourse.bass_utils import run_bass_kernel_spmd

    x = np.asarray(cin_inputs, dtype=np.float32)
    assert x.shape == (B_FULL, F, D)
    bf = ml_dtypes.bfloat16
    # [B, F, D] -> per-core [F, BS*D] bf16, tiled 4x along partitions
    xt_all = np.ascontiguousarray(
        x.reshape(N_CORES, BS, F, D).transpose(0, 2, 1, 3)
    ).astype(bf).reshape(N_CORES, F, BS * D)
    xt_all = np.ascontiguousarray(np.tile(xt_all, (1, 4, 1)))
    w0t = np.ascontiguousarray(np.asarray(w0, dtype=np.float32).T).astype(bf)
    w1t = np.ascontiguousarray(np.asarray(w1, dtype=np.float32).T).astype(bf)
    b0c = np.asarray(b0, dtype=np.float32).reshape(O, 1).copy()
    b1c = np.asarray(b1, dtype=np.float32).reshape(O, 1).copy()
    s1 = make_sels()

    nc = _get_nc()
    in_maps = []
    for i in range(N_CORES):
        in_maps.append(
            {
                "xt": xt_all[i],
                "xe0": np.ascontiguousarray(np.repeat(xt_all[i][0:32], 32, axis=0)),
                "w0t": w0t, "w1t": w1t,
                "sel1": s1, "b0": b0c, "b1": b1c,
            }
        )
    res = run_bass_kernel_spmd(nc, in_maps, core_ids=list(range(N_CORES)), trace=_trace)
    outs = []
    for r in res.results:
        o = np.concatenate([r["out0"], r["out1"]], axis=0).T
        outs.append(o)
    full = np.concatenate(outs, axis=0).astype(np.float32)
    if _trace:
        return full, res
    return full
